# revision 1
# baseline (speedup 1.0000x reference)
"""GNN message-passing (gather + segment_sum) Trainium2 kernel.

Reference semantics (full problem):
    out = segment_sum(x[src], dst, num_segments=50000)   x: [50000, 64] fp32
    edge_index: [2, 800000] (src; dst)

Sharding: destination nodes are range-partitioned over the 8 NeuronCores
(core c owns nodes [c*6250, (c+1)*6250)); each edge is routed to the core
owning its destination, so no cross-core reduction is needed. Each core
holds a full replica of the (hi|lo bf16-packed) node-feature table in HBM.

Device algorithm per core:
  - gpsimd/SWDGE dma_gather: msg[i] = x_pack[gather_idx[i]]   (HBM -> SBUF)
    x_pack rows are [bf16(x) | bf16(x - bf16(x))] (128 bf16 = 256 B), an
    exact fp32 split so the PE can run bf16 matmuls at full rate with
    ~1e-5 relative error.
  - vector/DVE: one-hot indicator per 128-edge group g:
        ind[p, n] = (dst_rel[p, g] == iota[n])    bf16 [128, 128]
    dummy tokens carry dst_rel = -1 -> all-zero indicator row.
  - tensor/PE: per group, accumulate into the destination node-tile's PSUM
    accumulator (all 49 tiles of [128 nodes, 64] live in PSUM at once):
        psum[tile] += ind.T @ msg_hi ;  psum[tile] += ind.T @ msg_lo
  - scalar/ACT: evacuate the 49 PSUM tiles to SBUF; gpsimd DMAs to HBM.

Host layout invariants (_build_layout):
  - edges sorted by (region = src<32768 ? lo : hi, dst tile, src); each
    (tile, region) block is padded to a multiple of 128 tokens so no
    128-token group spans two node tiles, and gather chunks within one
    region use a single table (int16 gather index limit).
  - per-(tile, region) column counts are maxed across all 8 cores so the
    SPMD instruction stream (PSUM offsets, start/stop flags) is identical.
  - token i of a dma_gather/SBUF grid lives at [i % 128, i // 128].
"""

import ml_dtypes
import numpy as np

import concourse.bacc as bacc
import concourse.mybir as mybir
from concourse.bass_utils import run_bass_kernel_spmd
from concourse.library_config import mlp

BF16 = ml_dtypes.bfloat16

N_NODES = 50000
N_EDGES = 800000
D = 64
PACK = 2 * D              # hi|lo packed row: 128 bf16 = 256 B
N_CORES = 8
NPC = N_NODES // N_CORES  # 6250 destination nodes per core
N_TILES = (NPC + 127) // 128  # 49 node tiles per core
OUT_ROWS = N_TILES * 128  # 6272
SPLIT = 32768             # int16 index limit for dma_gather
# SWDGE descriptor-ring capacity: ring_ndesc = scratch_bytes/16 per engine per
# side; a gather of n tokens needs n/16+1 descs and must fit in half the ring.
SCRATCH = 32768           # -> ring 2048, so chunks up to 16368 tokens
GCH = 96                  # chunk size in grid columns (12288 tokens, 769 descs)


def _build_layout(src, dst):
    """Place edges on per-core token grids; uniform across cores.

    Returns (cores, meta) where cores[c] has:
      gg [128, C] int32 gather index grid (lo region: src; hi: src-SPLIT; pad 0)
      dr [128, C] float dst_rel grid (node index within tile; pad -1)
    and meta has:
      C, c_lo, group_tiles [C], starts [C], stops [C]
    """
    src = np.asarray(src, np.int64)
    dst = np.asarray(dst, np.int64)
    core_of = dst // NPC
    # per (core, region, tile) edge lists
    buckets = {}
    cols = np.zeros((2, N_TILES), np.int64)  # [region, tile] -> max cols
    for c in range(N_CORES):
        sel = core_of == c
        s = src[sel]
        d = dst[sel] - c * NPC
        tile = d // 128
        rel = d % 128
        hi = (s >= SPLIT).astype(np.int64)
        for r in (0, 1):
            rs = hi == r
            for t in range(N_TILES):
                m = rs & (tile == t)
                buckets[(c, r, t)] = (s[m], rel[m])
                cols[r, t] = max(cols[r, t], -(-int(m.sum()) // 128))
    c_lo = int(cols[0].sum())
    C = int(cols.sum())
    group_tiles = []
    for r in (0, 1):
        for t in range(N_TILES):
            group_tiles += [t] * int(cols[r, t])
    group_tiles = np.array(group_tiles, np.int64)
    # PSUM start=True clears the has_written bits of the WHOLE bank, so it may
    # only be issued once per bank (on the bank's first matmul). With the bit
    # clear, a start=False matmul overwrites-and-marks; with it set, it
    # accumulates -- exactly per-region init-then-accumulate semantics.
    starts = np.zeros(C, bool)
    stops = np.zeros(C, bool)
    group_banks = group_tiles // 8
    for b in range(int(group_banks.max()) + 1):
        w = np.nonzero(group_banks == b)[0]
        starts[w[0]] = True
        stops[w[-1]] = True
    cores = []
    for c in range(N_CORES):
        gg = np.zeros((128, C), np.int32)
        dr = np.full((128, C), -1.0, np.float32)
        col0 = 0
        for r in (0, 1):
            for t in range(N_TILES):
                s, rel = buckets[(c, r, t)]
                k = np.arange(len(s))
                p = k % 128
                col = col0 + k // 128
                gg[p, col] = s - (SPLIT if r else 0)
                dr[p, col] = rel
                col0 += int(cols[r, t])
        cores.append({"gg": gg, "dr": dr})
    meta = {
        "C": C,
        "c_lo": c_lo,
        "group_tiles": group_tiles,
        "starts": starts,
        "stops": stops,
    }
    return cores, meta


def _grid_to_wrapped(grid):
    """[128, C] token grid -> [128, C*8] int16 wrapped index array.

    Token i lives at grid[i % 128, i // 128]; the SWDGE ucode reads token i
    from wrapped[i % 16, i // 16], replicated to all 8 Q7 cpu partition
    groups (rows 16k..16k+15).
    """
    P, C = grid.shape
    assert P == 128
    tok = grid.T.reshape(-1)
    return np.tile(tok.reshape(-1, 16).T.astype(np.int16), (8, 1))


def _chunks(meta):
    """Chunks of <= GCH columns, not crossing the lo/hi region boundary.

    Returns list of (table_idx, col_a, col_b)."""
    out = []
    for lo, hi_, tab in ((0, meta["c_lo"], 0), (meta["c_lo"], meta["C"], 1)):
        for a in range(lo, hi_, GCH):
            out.append((tab, a, min(a + GCH, hi_)))
    return out


def _build_nc(meta):
    C = meta["C"]
    chunks = _chunks(meta)
    n_ch = len(chunks)
    group_tiles = meta["group_tiles"]
    starts = meta["starts"]
    stops = meta["stops"]

    nc = bacc.Bacc("TRN2", dynamic_dma_scratch_size=SCRATCH)
    x_lo = nc.dram_tensor("x_lo", [SPLIT, PACK], mybir.dt.bfloat16, kind="ExternalInput")
    x_hi = nc.dram_tensor(
        "x_hi", [N_NODES - SPLIT, PACK], mybir.dt.bfloat16, kind="ExternalInput"
    )
    gg = nc.dram_tensor("gg", [128, 8 * C], mybir.dt.int16, kind="ExternalInput")
    dr = nc.dram_tensor("dr", [128, C], mybir.dt.float32, kind="ExternalInput")
    iot = nc.dram_tensor("iot", [128, 128], mybir.dt.bfloat16, kind="ExternalInput")
    out = nc.dram_tensor("out", [OUT_ROWS, D], mybir.dt.float32, kind="ExternalOutput")

    with (
        nc.Block() as block,
        nc.sbuf_tensor("msg0", [128, GCH, PACK], mybir.dt.bfloat16) as msg0,
        nc.sbuf_tensor("msg1", [128, GCH, PACK], mybir.dt.bfloat16) as msg1,
        nc.sbuf_tensor("ind0", [128, GCH * 128], mybir.dt.bfloat16) as ind0,
        nc.sbuf_tensor("ind1", [128, GCH * 128], mybir.dt.bfloat16) as ind1,
        nc.sbuf_tensor("t_gg", [128, 8 * C], mybir.dt.int16) as t_gg,
        nc.sbuf_tensor("t_dr", [128, C], mybir.dt.float32) as t_dr,
        nc.sbuf_tensor("t_iot", [128, 128], mybir.dt.bfloat16) as t_iot,
        nc.sbuf_tensor("outbuf", [128, N_TILES * D], mybir.dt.float32) as outbuf,
        nc.psum_tensor("acc", [128, 4096], mybir.dt.float32) as acc,
        nc.semaphore("up") as up,
        nc.semaphore("gs0") as gs0,
        nc.semaphore("gs1") as gs1,
        nc.semaphore("gs2") as gs2,
        nc.semaphore("gs3") as gs3,
        nc.semaphore("isem") as isem,
        nc.semaphore("psem") as psem,
        nc.semaphore("esem") as esem,
        nc.semaphore("osem") as osem,
    ):
        msgs = [msg0, msg1]
        inds = [ind0, ind1]
        gsems = [gs0, gs1, gs2, gs3]

        @block.gpsimd
        def _(g):
            g.load_library(mlp)
            g.dma_start(t_gg[:, :], gg[:, :]).then_inc(up, 16)
            g.dma_start(t_dr[:, :], dr[:, :]).then_inc(up, 16)
            g.dma_start(t_iot[:, :], iot[:, :]).then_inc(up, 16)
            g.wait_ge(up, 48)
            for k, (tab, a, b) in enumerate(chunks):
                ncols = b - a
                n = 128 * ncols
                if k >= 2:
                    g.wait_ge(psem, k - 1)
                table = x_lo if tab == 0 else x_hi
                g.dma_gather(
                    msgs[k % 2][:, :ncols, :],
                    table[:, :],
                    t_gg[:, 8 * a : 8 * b],
                    n,
                    n,
                    PACK,
                    single_packet=False,
                ).then_inc(gsems[k % 4], 16)
            g.wait_ge(esem, 1)
            out_v = out[:, :].rearrange("(a p) d -> p a d", p=128)
            ob_v = outbuf[:, :].rearrange("p (a d) -> p a d", a=N_TILES)
            g.dma_start(out_v, ob_v).then_inc(osem, 16)
            g.wait_ge(osem, 16)

        @block.vector
        def _(v):
            v.wait_ge(up, 48)
            for k, (tab, a, b) in enumerate(chunks):
                if k >= 2:
                    v.wait_ge(psem, k - 1)
                for j in range(b - a):
                    ins = v.tensor_scalar(
                        inds[k % 2][:, 128 * j : 128 * (j + 1)],
                        t_iot[:, :],
                        t_dr[:, a + j : a + j + 1],
                        None,
                        mybir.AluOpType.is_equal,
                    )
                ins.then_inc(isem, 1)

        @block.tensor
        def _(t):
            for k, (tab, a, b) in enumerate(chunks):
                t.wait_ge(gsems[k % 4], 16 * (k // 4 + 1))
                t.wait_ge(isem, k + 1)
                for j in range(b - a):
                    gidx = a + j
                    tl = int(group_tiles[gidx])
                    lhsT = inds[k % 2][:, 128 * j : 128 * (j + 1)]
                    mm1 = t.matmul(
                        acc[:, D * tl : D * (tl + 1)],
                        lhsT,
                        msgs[k % 2][:, j, 0:D],
                        start=bool(starts[gidx]),
                        stop=False,
                        skip_group_check=True,
                    )
                    mm2 = t.matmul(
                        acc[:, D * tl : D * (tl + 1)],
                        lhsT,
                        msgs[k % 2][:, j, D:PACK],
                        start=False,
                        stop=bool(stops[gidx]),
                        skip_group_check=True,
                    )
                mm2.then_inc(psem, 1)

        @block.scalar
        def _(s):
            s.wait_ge(psem, n_ch)
            for tl in range(N_TILES):
                ins = s.copy(
                    outbuf[:, D * tl : D * (tl + 1)], acc[:, D * tl : D * (tl + 1)]
                )
            ins.then_inc(esem, 1)

    nc.compile()
    return nc


_NC_CACHE = {}


def _get_nc(meta):
    key = (
        meta["C"],
        meta["c_lo"],
        meta["group_tiles"].tobytes(),
        meta["starts"].tobytes(),
        meta["stops"].tobytes(),
    )
    if key not in _NC_CACHE:
        _NC_CACHE[key] = _build_nc(meta)
    return _NC_CACHE[key]


def _pack_table(x):
    hi = x.astype(BF16)
    lo = (x - hi.astype(np.float32)).astype(BF16)
    return np.ascontiguousarray(np.concatenate([hi, lo], axis=1))


def kernel_with_result(x, edge_index, trace=False):
    x = np.ascontiguousarray(np.asarray(x, dtype=np.float32))
    ei = np.asarray(edge_index)
    assert x.shape == (N_NODES, D), x.shape
    cores, meta = _build_layout(ei[0], ei[1])
    nc = _get_nc(meta)
    xp = _pack_table(x)
    x_lo = np.ascontiguousarray(xp[:SPLIT])
    x_hi = np.ascontiguousarray(xp[SPLIT:])
    iot = np.tile(np.arange(128, dtype=np.float32).astype(BF16), (128, 1))
    in_maps = [
        {
            "x_lo": x_lo,
            "x_hi": x_hi,
            "gg": _grid_to_wrapped(info["gg"]),
            "dr": np.ascontiguousarray(info["dr"].astype(np.float32)),
            "iot": iot,
        }
        for info in cores
    ]
    res = run_bass_kernel_spmd(nc, in_maps, core_ids=list(range(N_CORES)), trace=trace)
    out = np.concatenate([r["out"][:NPC] for r in res.results], axis=0)
    return out, res


def kernel(x, edge_index):
    out, _ = kernel_with_result(x, edge_index)
    return out



# revision 3
# speedup vs baseline: 1.8601x; 1.8601x over previous
"""GNN message-passing (gather + segment_sum) Trainium2 kernel.

Reference semantics (full problem):
    out = segment_sum(x[src], dst, num_segments=50000)   x: [50000, 64] fp32
    edge_index: [2, 800000] (src; dst)

Sharding: destination nodes are range-partitioned over the 8 NeuronCores
(core c owns nodes [c*6250, (c+1)*6250)); each edge is routed to the core
owning its destination, so no cross-core reduction is needed. Each core
holds a full replica of the (hi|lo bf16-packed) node-feature table in HBM.

Device algorithm per core:
  - gpsimd/SWDGE dma_gather: msg[i] = x_pack[gather_idx[i]]   (HBM -> SBUF)
    x_pack rows are [bf16(x) | bf16(x - bf16(x))] (128 bf16 = 256 B), an
    exact fp32 split so the PE can run bf16 matmuls at full rate with
    ~1e-5 relative error. Gather chunks round-robin over all 4 SWDGE
    queues: each queue's descriptor generation runs on its own Q7 cpu
    pair, so desc-gen for different queues can overlap.
  - vector/DVE: one batched is_equal per chunk builds the one-hot
    indicators for all its 128-edge groups at once:
        ind[p, j, n] = (dst_rel[p, a+j] == iota[n])   bf16 [128, ncols, 128]
    via stride-0 broadcast APs (dr replicated along n, iota along j).
    Dummy tokens carry dst_rel = -1 -> all-zero indicator row.
  - tensor/PE: per group, accumulate into the destination node-tile's PSUM
    accumulator (all 49 tiles of [128 nodes, 64] live in PSUM at once):
        psum[tile] += ind.T @ msg_hi ;  psum[tile] += ind.T @ msg_lo
  - scalar/ACT: evacuate the 49 PSUM tiles to SBUF; gpsimd DMAs to HBM.

Host layout invariants (_build_layout):
  - edges sorted by (region = src<32768 ? lo : hi, dst tile, src); each
    (tile, region) block is padded to a multiple of 128 tokens so no
    128-token group spans two node tiles, and gather chunks within one
    region use a single table (int16 gather index limit).
  - per-(tile, region) column counts are maxed across all 8 cores so the
    SPMD instruction stream (PSUM offsets, start/stop flags) is identical.
  - token i of a dma_gather/SBUF grid lives at [i % 128, i // 128].
"""

import ml_dtypes
import numpy as np

import concourse.bacc as bacc
import concourse.mybir as mybir
from concourse.bass_utils import run_bass_kernel_spmd
from concourse.library_config import mlp

BF16 = ml_dtypes.bfloat16

N_NODES = 50000
N_EDGES = 800000
D = 64
PACK = 2 * D              # hi|lo packed row: 128 bf16 = 256 B
N_CORES = 8
NPC = N_NODES // N_CORES  # 6250 destination nodes per core
N_TILES = (NPC + 127) // 128  # 49 node tiles per core
OUT_ROWS = N_TILES * 128  # 6272
SPLIT = 32768             # int16 index limit for dma_gather
# SWDGE descriptor-ring capacity: ring_ndesc = scratch_bytes/16 per queue per
# side; a gather of n tokens needs n/16+1 descs per engine and must fit the
# ring with the in-flight chunks of that queue.
SCRATCH = 32768           # -> ring 2048 descs per queue
GCH = 48                  # chunk size in grid columns (6144 tokens, 385 descs)
NQ = 4                    # SWDGE queues; chunk k runs on queue k % NQ
NBUF = 4                  # msg/ind buffer sets; chunk k uses set k % NBUF


def _build_layout(src, dst):
    """Place edges on per-core token grids; uniform across cores.

    Returns (cores, meta) where cores[c] has:
      gg [128, C] int32 gather index grid (lo region: src; hi: src-SPLIT; pad 0)
      dr [128, C] float dst_rel grid (node index within tile; pad -1)
    and meta has:
      C, c_lo, group_tiles [C], starts [C], stops [C]
    """
    src = np.asarray(src, np.int64)
    dst = np.asarray(dst, np.int64)
    core_of = dst // NPC
    # per (core, region, tile) edge lists
    buckets = {}
    cols = np.zeros((2, N_TILES), np.int64)  # [region, tile] -> max cols
    for c in range(N_CORES):
        sel = core_of == c
        s = src[sel]
        d = dst[sel] - c * NPC
        tile = d // 128
        rel = d % 128
        hi = (s >= SPLIT).astype(np.int64)
        for r in (0, 1):
            rs = hi == r
            for t in range(N_TILES):
                m = rs & (tile == t)
                buckets[(c, r, t)] = (s[m], rel[m])
                cols[r, t] = max(cols[r, t], -(-int(m.sum()) // 128))
    c_lo = int(cols[0].sum())
    C = int(cols.sum())
    group_tiles = []
    for r in (0, 1):
        for t in range(N_TILES):
            group_tiles += [t] * int(cols[r, t])
    group_tiles = np.array(group_tiles, np.int64)
    # PSUM start=True clears the has_written bits of the WHOLE bank, so it may
    # only be issued once per bank (on the bank's first matmul). With the bit
    # clear, a start=False matmul overwrites-and-marks; with it set, it
    # accumulates -- exactly per-region init-then-accumulate semantics.
    starts = np.zeros(C, bool)
    stops = np.zeros(C, bool)
    group_banks = group_tiles // 8
    for b in range(int(group_banks.max()) + 1):
        w = np.nonzero(group_banks == b)[0]
        starts[w[0]] = True
        stops[w[-1]] = True
    cores = []
    for c in range(N_CORES):
        gg = np.zeros((128, C), np.int32)
        dr = np.full((128, C), -1.0, np.float32)
        col0 = 0
        for r in (0, 1):
            for t in range(N_TILES):
                s, rel = buckets[(c, r, t)]
                k = np.arange(len(s))
                p = k % 128
                col = col0 + k // 128
                gg[p, col] = s - (SPLIT if r else 0)
                dr[p, col] = rel
                col0 += int(cols[r, t])
        cores.append({"gg": gg, "dr": dr})
    meta = {
        "C": C,
        "c_lo": c_lo,
        "group_tiles": group_tiles,
        "starts": starts,
        "stops": stops,
    }
    return cores, meta


def _grid_to_wrapped(grid):
    """[128, C] token grid -> [128, C*8] int16 wrapped index array.

    Token i lives at grid[i % 128, i // 128]; the SWDGE ucode reads token i
    from wrapped[i % 16, i // 16], replicated to all 8 Q7 cpu partition
    groups (rows 16k..16k+15) so any queue's cpu pair finds them.
    """
    P, C = grid.shape
    assert P == 128
    tok = grid.T.reshape(-1)
    return np.tile(tok.reshape(-1, 16).T.astype(np.int16), (8, 1))


def _chunks(meta):
    """Chunks of <= GCH columns, not crossing the lo/hi region boundary.

    Returns list of (table_idx, col_a, col_b)."""
    out = []
    for lo, hi_, tab in ((0, meta["c_lo"], 0), (meta["c_lo"], meta["C"], 1)):
        for a in range(lo, hi_, GCH):
            out.append((tab, a, min(a + GCH, hi_)))
    return out


def _build_nc(meta):
    C = meta["C"]
    chunks = _chunks(meta)
    n_ch = len(chunks)
    group_tiles = meta["group_tiles"]
    starts = meta["starts"]
    stops = meta["stops"]

    nc = bacc.Bacc(
        "TRN2", dynamic_dma_scratch_size=SCRATCH, num_swdge_queues=NQ
    )
    x_lo = nc.dram_tensor("x_lo", [SPLIT, PACK], mybir.dt.bfloat16, kind="ExternalInput")
    x_hi = nc.dram_tensor(
        "x_hi", [N_NODES - SPLIT, PACK], mybir.dt.bfloat16, kind="ExternalInput"
    )
    gg = nc.dram_tensor("gg", [128, 8 * C], mybir.dt.int16, kind="ExternalInput")
    dr = nc.dram_tensor("dr", [128, C], mybir.dt.bfloat16, kind="ExternalInput")
    iot = nc.dram_tensor("iot", [128, 128], mybir.dt.bfloat16, kind="ExternalInput")
    out = nc.dram_tensor("out", [OUT_ROWS, D], mybir.dt.float32, kind="ExternalOutput")

    from contextlib import ExitStack

    with ExitStack() as stack:
        ec = stack.enter_context
        block = ec(nc.Block())
        msgs = [
            ec(nc.sbuf_tensor(f"msg{i}", [128, GCH, PACK], mybir.dt.bfloat16))
            for i in range(NBUF)
        ]
        inds = [
            ec(nc.sbuf_tensor(f"ind{i}", [128, GCH * 128], mybir.dt.bfloat16))
            for i in range(NBUF)
        ]
        t_gg = ec(nc.sbuf_tensor("t_gg", [128, 8 * C], mybir.dt.int16))
        t_dr = ec(nc.sbuf_tensor("t_dr", [128, C], mybir.dt.bfloat16))
        t_iot = ec(nc.sbuf_tensor("t_iot", [128, 128], mybir.dt.bfloat16))
        outbuf = ec(nc.sbuf_tensor("outbuf", [128, N_TILES * D], mybir.dt.float32))
        acc = ec(nc.psum_tensor("acc", [128, 4096], mybir.dt.float32))
        up = ec(nc.semaphore("up"))
        gsems = [ec(nc.semaphore(f"gs{i}")) for i in range(NQ)]
        isem = ec(nc.semaphore("isem"))
        psem = ec(nc.semaphore("psem"))
        esem = ec(nc.semaphore("esem"))
        osem = ec(nc.semaphore("osem"))

        @block.gpsimd
        def _(g):
            g.load_library(mlp)
            g.dma_start(t_gg[:, :], gg[:, :]).then_inc(up, 16)
            g.dma_start(t_dr[:, :], dr[:, :]).then_inc(up, 16)
            g.dma_start(t_iot[:, :], iot[:, :]).then_inc(up, 16)
            g.wait_ge(up, 48)
            for k, (tab, a, b) in enumerate(chunks):
                ncols = b - a
                n = 128 * ncols
                if k >= NBUF:
                    g.wait_ge(psem, k - NBUF + 1)
                table = x_lo if tab == 0 else x_hi
                g.dma_gather(
                    msgs[k % NBUF][:, :ncols, :],
                    table[:, :],
                    t_gg[:, 8 * a : 8 * b],
                    n,
                    n,
                    PACK,
                    single_packet=False,
                    queue_num=k % NQ,
                ).then_inc(gsems[k % NQ], 16)
            g.wait_ge(esem, 1)
            out_v = out[:, :].rearrange("(a p) d -> p a d", p=128)
            ob_v = outbuf[:, :].rearrange("p (a d) -> p a d", a=N_TILES)
            g.dma_start(out_v, ob_v).then_inc(osem, 16)
            g.wait_ge(osem, 16)

        @block.vector
        def _(v):
            v.wait_ge(up, 48)
            for k, (tab, a, b) in enumerate(chunks):
                ncols = b - a
                if k >= NBUF:
                    v.wait_ge(psem, k - NBUF + 1)
                out_ap = inds[k % NBUF][:, : ncols * 128].rearrange(
                    "p (a b) -> p a b", a=ncols
                )
                dr_rep = (
                    t_dr[:, a:b].unsqueeze(2).broadcast_to([128, ncols, 128])
                )
                iot_rep = t_iot[:, :].unsqueeze(1).broadcast_to([128, ncols, 128])
                ins = v.tensor_tensor(
                    out_ap, dr_rep, iot_rep, mybir.AluOpType.is_equal
                )
                ins.then_inc(isem, 1)

        @block.tensor
        def _(t):
            nq_seen = [0] * NQ
            for k, (tab, a, b) in enumerate(chunks):
                nq_seen[k % NQ] += 1
                t.wait_ge(gsems[k % NQ], 16 * nq_seen[k % NQ])
                t.wait_ge(isem, k + 1)
                for j in range(b - a):
                    gidx = a + j
                    tl = int(group_tiles[gidx])
                    lhsT = inds[k % NBUF][:, 128 * j : 128 * (j + 1)]
                    mm1 = t.matmul(
                        acc[:, D * tl : D * (tl + 1)],
                        lhsT,
                        msgs[k % NBUF][:, j, 0:D],
                        start=bool(starts[gidx]),
                        stop=False,
                        skip_group_check=True,
                    )
                    mm2 = t.matmul(
                        acc[:, D * tl : D * (tl + 1)],
                        lhsT,
                        msgs[k % NBUF][:, j, D:PACK],
                        start=False,
                        stop=bool(stops[gidx]),
                        skip_group_check=True,
                    )
                mm2.then_inc(psem, 1)

        @block.scalar
        def _(s):
            s.wait_ge(psem, n_ch)
            for tl in range(N_TILES):
                ins = s.copy(
                    outbuf[:, D * tl : D * (tl + 1)], acc[:, D * tl : D * (tl + 1)]
                )
            ins.then_inc(esem, 1)

    nc.compile()
    return nc


_NC_CACHE = {}


def _get_nc(meta):
    key = (
        meta["C"],
        meta["c_lo"],
        meta["group_tiles"].tobytes(),
        meta["starts"].tobytes(),
        meta["stops"].tobytes(),
    )
    if key not in _NC_CACHE:
        _NC_CACHE[key] = _build_nc(meta)
    return _NC_CACHE[key]


def _pack_table(x):
    hi = x.astype(BF16)
    lo = (x - hi.astype(np.float32)).astype(BF16)
    return np.ascontiguousarray(np.concatenate([hi, lo], axis=1))


def kernel_with_result(x, edge_index, trace=False):
    x = np.ascontiguousarray(np.asarray(x, dtype=np.float32))
    ei = np.asarray(edge_index)
    assert x.shape == (N_NODES, D), x.shape
    cores, meta = _build_layout(ei[0], ei[1])
    nc = _get_nc(meta)
    xp = _pack_table(x)
    x_lo = np.ascontiguousarray(xp[:SPLIT])
    x_hi = np.ascontiguousarray(xp[SPLIT:])
    iot = np.tile(np.arange(128, dtype=np.float32).astype(BF16), (128, 1))
    in_maps = [
        {
            "x_lo": x_lo,
            "x_hi": x_hi,
            "gg": _grid_to_wrapped(info["gg"]),
            "dr": np.ascontiguousarray(info["dr"].astype(BF16)),
            "iot": iot,
        }
        for info in cores
    ]
    res = run_bass_kernel_spmd(nc, in_maps, core_ids=list(range(N_CORES)), trace=trace)
    out = np.concatenate([r["out"][:NPC] for r in res.results], axis=0)
    return out, res


def kernel(x, edge_index):
    out, _ = kernel_with_result(x, edge_index)
    return out


# revision 4
# speedup vs baseline: 2.3419x; 1.2590x over previous
"""GNN message-passing (gather + segment_sum) Trainium2 kernel.

Reference semantics (full problem):
    out = segment_sum(x[src], dst, num_segments=50000)   x: [50000, 64] fp32
    edge_index: [2, 800000] (src; dst)

Sharding: destination nodes are range-partitioned over the 8 NeuronCores
(core c owns nodes [c*6250, (c+1)*6250)); each edge is routed to the core
owning its destination, so no cross-core reduction is needed. Each core
holds a full replica of the (hi|lo bf16-packed) node-feature table in HBM.

Device algorithm per core:
  - gpsimd/SWDGE dma_gather: msg[i] = x_pack[gather_idx[i]]   (HBM -> SBUF)
    x_pack rows are [bf16(x) | bf16(x - bf16(x))] (128 bf16 = 256 B), an
    exact fp32 split so the PE can run bf16 matmuls at full rate with
    ~1e-5 relative error. Gather chunks round-robin over all 4 SWDGE
    queues: each queue's descriptor generation runs on its own Q7 cpu
    pair, so desc-gen for different queues overlaps; a 5-deep buffer ring
    lets the next wave's desc-gen overlap the previous wave's transfers.
  - vector/DVE: one batched is_equal per chunk builds the one-hot
    indicators for all its 128-edge groups at once:
        ind[p, j, n] = (dst_rel[p, a+j] == iota[n])   bf16 [128, ncols, 128]
    via stride-0 broadcast APs (dr replicated along n, iota along j).
    Dummy tokens carry dst_rel = -1 -> all-zero indicator row.
  - tensor/PE: per group, accumulate into the destination node-tile's PSUM
    accumulator (all 49 tiles of [128 nodes, 64] live in PSUM at once):
        psum[tile] += ind.T @ msg_hi ;  psum[tile] += ind.T @ msg_lo
  - scalar/ACT: evacuate each PSUM bank to SBUF as soon as its last group
    has accumulated; sync-engine HWDGE DMAs handle the input uploads and
    the final output store.

Host layout invariants (_build_layout):
  - edges sorted by (region = src<32768 ? lo : hi, dst tile, src); each
    (tile, region) block is padded to a multiple of 128 tokens so no
    128-token group spans two node tiles, and gather chunks within one
    region use a single table (int16 gather index limit).
  - per-(tile, region) column counts are maxed across all 8 cores so the
    SPMD instruction stream (PSUM offsets, start/stop flags) is identical.
  - token i of a dma_gather/SBUF grid lives at [i % 128, i // 128].
"""

from contextlib import ExitStack

import ml_dtypes
import numpy as np

import concourse.bacc as bacc
import concourse.mybir as mybir
from concourse.bass_utils import run_bass_kernel_spmd
from concourse.library_config import mlp

BF16 = ml_dtypes.bfloat16

N_NODES = 50000
N_EDGES = 800000
D = 64
PACK = 2 * D              # hi|lo packed row: 128 bf16 = 256 B
N_CORES = 8
NPC = N_NODES // N_CORES  # 6250 destination nodes per core
N_TILES = (NPC + 127) // 128  # 49 node tiles per core
N_BANKS = (N_TILES + 7) // 8  # 7 PSUM banks hold the 49 tiles
OUT_ROWS = N_TILES * 128  # 6272
SPLIT = 32768             # int16 index limit for dma_gather
# SWDGE descriptor-ring capacity: ring_ndesc = scratch_bytes/16 per queue per
# side; a gather of n tokens needs n/16+1 descs per engine and must fit the
# ring with the in-flight chunks of that queue.
SCRATCH = 32768           # -> ring 2048 descs per queue
GCH = 48                  # max chunk size in grid columns
NQ = 4                    # SWDGE queues; chunk k runs on queue k % NQ
NBUF = 5                  # msg/ind buffer sets; chunk k uses set k % NBUF


def _build_layout(src, dst):
    """Place edges on per-core token grids; uniform across cores.

    Returns (cores, meta) where cores[c] has:
      gg [128, C] int32 gather index grid (lo region: src; hi: src-SPLIT; pad 0)
      dr [128, C] float dst_rel grid (node index within tile; pad -1)
    and meta has:
      C, c_lo, group_tiles [C], starts [C], stops [C]
    """
    src = np.asarray(src, np.int64)
    dst = np.asarray(dst, np.int64)
    core_of = dst // NPC
    # per (core, region, tile) edge lists
    buckets = {}
    cols = np.zeros((2, N_TILES), np.int64)  # [region, tile] -> max cols
    for c in range(N_CORES):
        sel = core_of == c
        s = src[sel]
        d = dst[sel] - c * NPC
        tile = d // 128
        rel = d % 128
        hi = (s >= SPLIT).astype(np.int64)
        for r in (0, 1):
            rs = hi == r
            for t in range(N_TILES):
                m = rs & (tile == t)
                buckets[(c, r, t)] = (s[m], rel[m])
                cols[r, t] = max(cols[r, t], -(-int(m.sum()) // 128))
    c_lo = int(cols[0].sum())
    C = int(cols.sum())
    group_tiles = []
    for r in (0, 1):
        for t in range(N_TILES):
            group_tiles += [t] * int(cols[r, t])
    group_tiles = np.array(group_tiles, np.int64)
    # PSUM start=True clears the has_written bits of the WHOLE bank, so it may
    # only be issued once per bank (on the bank's first matmul). With the bit
    # clear, a start=False matmul overwrites-and-marks; with it set, it
    # accumulates -- exactly per-region init-then-accumulate semantics.
    starts = np.zeros(C, bool)
    stops = np.zeros(C, bool)
    group_banks = group_tiles // 8
    for b in range(int(group_banks.max()) + 1):
        w = np.nonzero(group_banks == b)[0]
        starts[w[0]] = True
        stops[w[-1]] = True
    cores = []
    for c in range(N_CORES):
        gg = np.zeros((128, C), np.int32)
        dr = np.full((128, C), -1.0, np.float32)
        col0 = 0
        for r in (0, 1):
            for t in range(N_TILES):
                s, rel = buckets[(c, r, t)]
                k = np.arange(len(s))
                p = k % 128
                col = col0 + k // 128
                gg[p, col] = s - (SPLIT if r else 0)
                dr[p, col] = rel
                col0 += int(cols[r, t])
        cores.append({"gg": gg, "dr": dr})
    meta = {
        "C": C,
        "c_lo": c_lo,
        "group_tiles": group_tiles,
        "starts": starts,
        "stops": stops,
    }
    return cores, meta


def _grid_to_wrapped(grid):
    """[128, C] token grid -> [128, C*8] int16 wrapped index array.

    Token i lives at grid[i % 128, i // 128]; the SWDGE ucode reads token i
    from wrapped[i % 16, i // 16], replicated to all 8 Q7 cpu partition
    groups (rows 16k..16k+15) so any queue's cpu pair finds them.
    """
    P, C = grid.shape
    assert P == 128
    tok = grid.T.reshape(-1)
    return np.tile(tok.reshape(-1, 16).T.astype(np.int16), (8, 1))


def _chunks(meta):
    """Balanced chunks of <= GCH columns, not crossing the lo/hi boundary.

    Returns list of (table_idx, col_a, col_b)."""
    out = []
    for lo, hi_, tab in ((0, meta["c_lo"], 0), (meta["c_lo"], meta["C"], 1)):
        n = hi_ - lo
        k = -(-n // GCH)
        edges = [lo + (n * i) // k for i in range(k)] + [hi_]
        for i in range(k):
            out.append((tab, edges[i], edges[i + 1]))
    return out


def _build_nc(meta):
    C = meta["C"]
    chunks = _chunks(meta)
    n_ch = len(chunks)
    group_tiles = meta["group_tiles"]
    starts = meta["starts"]
    stops = meta["stops"]
    # chunk index after which PSUM bank b is complete (its stop group done)
    bank_done_chunk = [0] * N_BANKS
    for b in range(N_BANKS):
        w = np.nonzero(group_tiles // 8 == b)[0]
        last = int(w[-1])
        for k, (_, a, bb) in enumerate(chunks):
            if a <= last < bb:
                bank_done_chunk[b] = k
                break

    nc = bacc.Bacc(
        "TRN2", dynamic_dma_scratch_size=SCRATCH, num_swdge_queues=NQ
    )
    x_lo = nc.dram_tensor("x_lo", [SPLIT, PACK], mybir.dt.bfloat16, kind="ExternalInput")
    x_hi = nc.dram_tensor(
        "x_hi", [N_NODES - SPLIT, PACK], mybir.dt.bfloat16, kind="ExternalInput"
    )
    gg = nc.dram_tensor("gg", [128, 8 * C], mybir.dt.int16, kind="ExternalInput")
    dr = nc.dram_tensor("dr", [128, C], mybir.dt.bfloat16, kind="ExternalInput")
    iot = nc.dram_tensor("iot", [128, 128], mybir.dt.bfloat16, kind="ExternalInput")
    out = nc.dram_tensor("out", [OUT_ROWS, D], mybir.dt.float32, kind="ExternalOutput")

    with ExitStack() as stack:
        ec = stack.enter_context
        block = ec(nc.Block())
        msgs = [
            ec(nc.sbuf_tensor(f"msg{i}", [128, GCH, PACK], mybir.dt.bfloat16))
            for i in range(NBUF)
        ]
        inds = [
            ec(nc.sbuf_tensor(f"ind{i}", [128, GCH * 128], mybir.dt.bfloat16))
            for i in range(NBUF)
        ]
        t_gg = ec(nc.sbuf_tensor("t_gg", [128, 8 * C], mybir.dt.int16))
        t_dr = ec(nc.sbuf_tensor("t_dr", [128, C], mybir.dt.bfloat16))
        t_iot = ec(nc.sbuf_tensor("t_iot", [128, 128], mybir.dt.bfloat16))
        outbuf = ec(nc.sbuf_tensor("outbuf", [128, N_TILES * D], mybir.dt.float32))
        acc = ec(nc.psum_tensor("acc", [128, 4096], mybir.dt.float32))
        up = ec(nc.semaphore("up"))
        gsems = [ec(nc.semaphore(f"gs{i}")) for i in range(NQ)]
        isem = ec(nc.semaphore("isem"))
        psem = ec(nc.semaphore("psem"))
        esem = ec(nc.semaphore("esem"))
        osem = ec(nc.semaphore("osem"))

        @block.sync
        def _(y):
            y.dma_start(t_gg[:, :], gg[:, :]).then_inc(up, 16)
            y.dma_start(t_dr[:, :], dr[:, :]).then_inc(up, 16)
            y.dma_start(t_iot[:, :], iot[:, :]).then_inc(up, 16)
            y.wait_ge(esem, N_BANKS)
            out_v = out[:, :].rearrange("(a p) d -> p a d", p=128)
            ob_v = outbuf[:, :].rearrange("p (a d) -> p a d", a=N_TILES)
            y.dma_start(out_v, ob_v).then_inc(osem, 16)
            y.wait_ge(osem, 16)

        @block.gpsimd
        def _(g):
            g.load_library(mlp)
            g.wait_ge(up, 16)  # t_gg uploaded
            for k, (tab, a, b) in enumerate(chunks):
                ncols = b - a
                n = 128 * ncols
                if k >= NBUF:
                    g.wait_ge(psem, k - NBUF + 1)
                table = x_lo if tab == 0 else x_hi
                g.dma_gather(
                    msgs[k % NBUF][:, :ncols, :],
                    table[:, :],
                    t_gg[:, 8 * a : 8 * b],
                    n,
                    n,
                    PACK,
                    single_packet=False,
                    queue_num=k % NQ,
                ).then_inc(gsems[k % NQ], 16)

        @block.vector
        def _(v):
            v.wait_ge(up, 48)
            for k, (tab, a, b) in enumerate(chunks):
                ncols = b - a
                if k >= NBUF:
                    v.wait_ge(psem, k - NBUF + 1)
                out_ap = inds[k % NBUF][:, : ncols * 128].rearrange(
                    "p (a b) -> p a b", a=ncols
                )
                dr_rep = (
                    t_dr[:, a:b].unsqueeze(2).broadcast_to([128, ncols, 128])
                )
                iot_rep = t_iot[:, :].unsqueeze(1).broadcast_to([128, ncols, 128])
                ins = v.tensor_tensor(
                    out_ap, dr_rep, iot_rep, mybir.AluOpType.is_equal
                )
                ins.then_inc(isem, 1)

        @block.tensor
        def _(t):
            nq_seen = [0] * NQ
            for k, (tab, a, b) in enumerate(chunks):
                nq_seen[k % NQ] += 1
                t.wait_ge(gsems[k % NQ], 16 * nq_seen[k % NQ])
                t.wait_ge(isem, k + 1)
                for j in range(b - a):
                    gidx = a + j
                    tl = int(group_tiles[gidx])
                    lhsT = inds[k % NBUF][:, 128 * j : 128 * (j + 1)]
                    mm1 = t.matmul(
                        acc[:, D * tl : D * (tl + 1)],
                        lhsT,
                        msgs[k % NBUF][:, j, 0:D],
                        start=bool(starts[gidx]),
                        stop=False,
                        skip_group_check=True,
                    )
                    mm2 = t.matmul(
                        acc[:, D * tl : D * (tl + 1)],
                        lhsT,
                        msgs[k % NBUF][:, j, D:PACK],
                        start=False,
                        stop=bool(stops[gidx]),
                        skip_group_check=True,
                    )
                mm2.then_inc(psem, 1)

        @block.scalar
        def _(s):
            for b in range(N_BANKS):
                s.wait_ge(psem, bank_done_chunk[b] + 1)
                t0, t1 = 8 * b, min(8 * b + 8, N_TILES)
                ins = s.copy(
                    outbuf[:, D * t0 : D * t1], acc[:, D * t0 : D * t1]
                )
                ins.then_inc(esem, 1)

    nc.compile()
    return nc


_NC_CACHE = {}


def _get_nc(meta):
    key = (
        meta["C"],
        meta["c_lo"],
        meta["group_tiles"].tobytes(),
        meta["starts"].tobytes(),
        meta["stops"].tobytes(),
    )
    if key not in _NC_CACHE:
        _NC_CACHE[key] = _build_nc(meta)
    return _NC_CACHE[key]


def _pack_table(x):
    hi = x.astype(BF16)
    lo = (x - hi.astype(np.float32)).astype(BF16)
    return np.ascontiguousarray(np.concatenate([hi, lo], axis=1))


def kernel_with_result(x, edge_index, trace=False):
    x = np.ascontiguousarray(np.asarray(x, dtype=np.float32))
    ei = np.asarray(edge_index)
    assert x.shape == (N_NODES, D), x.shape
    cores, meta = _build_layout(ei[0], ei[1])
    nc = _get_nc(meta)
    xp = _pack_table(x)
    x_lo = np.ascontiguousarray(xp[:SPLIT])
    x_hi = np.ascontiguousarray(xp[SPLIT:])
    iot = np.tile(np.arange(128, dtype=np.float32).astype(BF16), (128, 1))
    in_maps = [
        {
            "x_lo": x_lo,
            "x_hi": x_hi,
            "gg": _grid_to_wrapped(info["gg"]),
            "dr": np.ascontiguousarray(info["dr"].astype(BF16)),
            "iot": iot,
        }
        for info in cores
    ]
    res = run_bass_kernel_spmd(nc, in_maps, core_ids=list(range(N_CORES)), trace=trace)
    out = np.concatenate([r["out"][:NPC] for r in res.results], axis=0)
    return out, res


def kernel(x, edge_index):
    out, _ = kernel_with_result(x, edge_index)
    return out


# revision 7
# speedup vs baseline: 3.0250x; 1.2917x over previous
"""GNN message-passing (gather + segment_sum) Trainium2 kernel.

Reference semantics (full problem):
    out = segment_sum(x[src], dst, num_segments=50000)   x: [50000, 64] fp32
    edge_index: [2, 800000] (src; dst)

Sharding: destination nodes are range-partitioned over the 8 NeuronCores
(core c owns nodes [c*6250, (c+1)*6250)); each edge is routed to the core
owning its destination, so no cross-core reduction is needed. Each core
holds a full replica of the bf16 node-feature table in HBM (256 B rows;
only the first 128 B - the bf16 feature row - is fetched per edge).

Device algorithm per core:
  - gpsimd/SWDGE dma_gather: msg[i] = x_bf16[gather_idx[i]]  (HBM -> SBUF)
    emitted as a raw InstDMAGatherAnt with elem_size=64 bf16 (128 B) and
    elem_step=128 (256 B row stride): the non-transpose ucode path allows
    sub-256B elements as long as the row stride is a 256 B multiple.
    bf16 messages + fp32 PSUM accumulation give ~2e-3 relative error
    (graded gate is 2e-2). Gather chunks round-robin over all 4 SWDGE
    queues: each queue's descriptor generation runs on its own Q7 cpu
    pair, so desc-gen for different queues overlaps; a 6-deep buffer ring
    lets the next chunks' desc-gen overlap earlier chunks' transfers.
    Each buffer slot has its own DMA-completion semaphore: with two
    gathers of one queue in flight, a shared per-queue semaphore would
    alias their 16 per-engine completion increments.
  - vector/DVE: one batched is_equal per chunk builds the one-hot
    indicators for all its 128-edge groups at once:
        ind[p, j, n] = (dst_rel[p, a+j] == iota[n])   bf16 [128, ncols, 128]
    via stride-0 broadcast APs (dr replicated along n, iota along j).
    Dummy tokens carry dst_rel = -1 -> all-zero indicator row.
  - tensor/PE: per group, accumulate into the destination node-tile's PSUM
    accumulator (all 49 tiles of [128 nodes, 64] live in PSUM at once):
        psum[tile] += ind.T @ msg
  - scalar/ACT: evacuate each PSUM bank to SBUF as soon as its last group
    has accumulated; sync-engine HWDGE DMAs handle the input uploads and
    the final output store.

Host layout invariants (_build_layout):
  - edges sorted by (region = src<32768 ? lo : hi, dst tile, src); each
    (tile, region) block is padded to a multiple of 128 tokens so no
    128-token group spans two node tiles, and gather chunks within one
    region use a single table (int16 gather index limit).
  - per-(tile, region) column counts are maxed across all 8 cores so the
    SPMD instruction stream (PSUM offsets, start/stop flags) is identical.
  - token i of a dma_gather/SBUF grid lives at [i % 128, i // 128].
"""

from contextlib import ExitStack

import ml_dtypes
import numpy as np

import concourse.bacc as bacc
import concourse.mybir as mybir
from concourse.bass_utils import run_bass_kernel_spmd
from concourse.library_config import mlp

BF16 = ml_dtypes.bfloat16

N_NODES = 50000
N_EDGES = 800000
D = 64
PACK = 2 * D              # table row pitch: 128 bf16 = 256 B (only 128 B used)
N_CORES = 8
NPC = N_NODES // N_CORES  # 6250 destination nodes per core
N_TILES = (NPC + 127) // 128  # 49 node tiles per core
N_BANKS = (N_TILES + 7) // 8  # 7 PSUM banks hold the 49 tiles
OUT_ROWS = N_TILES * 128  # 6272
SPLIT = 32768             # int16 index limit for dma_gather
# SWDGE descriptor-ring capacity: ring_ndesc = scratch_bytes/16 per queue per
# side; a gather of n tokens needs n/16+1 descs per engine and must fit the
# ring with the in-flight chunks of that queue.
SCRATCH = 32768           # -> ring 2048 descs per queue
GCH = 56                  # max chunk size in grid columns
NQ = 4                    # SWDGE queues; chunk k runs on queue k % NQ
NBUF = 6                  # msg/ind buffer sets; chunk k uses set k % NBUF


def _build_layout(src, dst):
    """Place edges on per-core token grids; uniform across cores.

    Returns (cores, meta) where cores[c] has:
      gg [128, C] int32 gather index grid (lo region: src; hi: src-SPLIT; pad 0)
      dr [128, C] float dst_rel grid (node index within tile; pad -1)
    and meta has:
      C, c_lo, group_tiles [C], starts [C], stops [C]
    """
    src = np.asarray(src, np.int64)
    dst = np.asarray(dst, np.int64)
    core_of = dst // NPC
    # per (core, region, tile) edge lists
    buckets = {}
    cols = np.zeros((2, N_TILES), np.int64)  # [region, tile] -> max cols
    for c in range(N_CORES):
        sel = core_of == c
        s = src[sel]
        d = dst[sel] - c * NPC
        tile = d // 128
        rel = d % 128
        hi = (s >= SPLIT).astype(np.int64)
        for r in (0, 1):
            rs = hi == r
            for t in range(N_TILES):
                m = rs & (tile == t)
                buckets[(c, r, t)] = (s[m], rel[m])
                cols[r, t] = max(cols[r, t], -(-int(m.sum()) // 128))
    c_lo = int(cols[0].sum())
    C = int(cols.sum())
    group_tiles = []
    for r in (0, 1):
        for t in range(N_TILES):
            group_tiles += [t] * int(cols[r, t])
    group_tiles = np.array(group_tiles, np.int64)
    # PSUM start=True clears the has_written bits of the WHOLE bank, so it may
    # only be issued once per bank (on the bank's first matmul). With the bit
    # clear, a start=False matmul overwrites-and-marks; with it set, it
    # accumulates -- exactly per-region init-then-accumulate semantics.
    starts = np.zeros(C, bool)
    stops = np.zeros(C, bool)
    group_banks = group_tiles // 8
    for b in range(int(group_banks.max()) + 1):
        w = np.nonzero(group_banks == b)[0]
        starts[w[0]] = True
        stops[w[-1]] = True
    cores = []
    for c in range(N_CORES):
        gg = np.zeros((128, C), np.int32)
        dr = np.full((128, C), -1.0, np.float32)
        col0 = 0
        for r in (0, 1):
            for t in range(N_TILES):
                s, rel = buckets[(c, r, t)]
                k = np.arange(len(s))
                p = k % 128
                col = col0 + k // 128
                gg[p, col] = s - (SPLIT if r else 0)
                dr[p, col] = rel
                col0 += int(cols[r, t])
        cores.append({"gg": gg, "dr": dr})
    meta = {
        "C": C,
        "c_lo": c_lo,
        "group_tiles": group_tiles,
        "starts": starts,
        "stops": stops,
    }
    return cores, meta


def _grid_to_wrapped(grid):
    """[128, C] token grid -> [128, C*8] int16 wrapped index array.

    Token i lives at grid[i % 128, i // 128]; the SWDGE ucode reads token i
    from wrapped[i % 16, i // 16], replicated to all 8 Q7 cpu partition
    groups (rows 16k..16k+15) so any queue's cpu pair finds them.
    """
    P, C = grid.shape
    assert P == 128
    tok = grid.T.reshape(-1)
    return np.tile(tok.reshape(-1, 16).T.astype(np.int16), (8, 1))


def _chunks(meta):
    """Balanced chunks of <= GCH columns, not crossing the lo/hi boundary.

    Returns list of (table_idx, col_a, col_b)."""
    out = []
    for lo, hi_, tab in ((0, meta["c_lo"], 0), (meta["c_lo"], meta["C"], 1)):
        n = hi_ - lo
        k = -(-n // GCH)
        edges = [lo + (n * i) // k for i in range(k)] + [hi_]
        for i in range(k):
            out.append((tab, edges[i], edges[i + 1]))
    return out


def _dma_gather_hi(g, nc, out_ap, in_ap, idxs_ap, num_idxs, queue_num):
    """Raw InstDMAGatherAnt: gather elem_size=64 bf16 (128 B) rows from a
    table with 256 B row pitch (elem_step=128). Mirrors bass.dma_gather's
    non-transpose DRAM lowering, which over-conservatively asserts
    elem%256B: the ucode only requires the row *stride* be a 256 B
    multiple (the %256 assert guards the transpose path)."""
    elem_size = D          # 64 bf16 = 128 B fetched per token
    elem_step = PACK       # 256 B row pitch
    stride_bytes = elem_step * 2
    assert stride_bytes % 256 == 0
    assert in_ap.ap[0][0] == elem_step, in_ap.ap
    assert in_ap.ap[-1][1] == elem_size, in_ap.ap
    assert out_ap.ap[-1][1] == elem_size, out_ap.ap
    assert out_ap.ap[0][1] * out_ap.ap[1][1] == num_idxs
    assert num_idxs % 128 == 0
    _in_ap = g.lower_ap_dma(in_ap, for_custom_bir_dma=True)
    _idxs_ap = g.lower_ap(idxs_ap)
    _out_ap = g.lower_ap(out_ap)
    return g.add_instruction(
        mybir.InstDMAGatherAnt(
            name=nc.get_next_instruction_name(),
            ins=[*_in_ap, _idxs_ap, g.lower_val_access(g.to_reg(num_idxs))],
            outs=[_out_ap],
            transpose=False,
            num_idxs=num_idxs,
            elem_size=elem_size,
            stride_bytes_256=stride_bytes // 256,
            gen_mode=0,
            single_packet=False,
            queue_num=queue_num,
            sbuf_tokens_per_rank=0,
            sbuf_free_dim_per_rank=0,
            sbuf_free_dim_pad_per_rank=0,
            sbuf_byte_offset=0,
        )
    )


def _build_nc(meta):
    C = meta["C"]
    chunks = _chunks(meta)
    n_ch = len(chunks)
    group_tiles = meta["group_tiles"]
    starts = meta["starts"]
    stops = meta["stops"]
    # chunk index after which PSUM bank b is complete (its stop group done)
    bank_done_chunk = [0] * N_BANKS
    for b in range(N_BANKS):
        w = np.nonzero(group_tiles // 8 == b)[0]
        last = int(w[-1])
        for k, (_, a, bb) in enumerate(chunks):
            if a <= last < bb:
                bank_done_chunk[b] = k
                break

    nc = bacc.Bacc(
        "TRN2", dynamic_dma_scratch_size=SCRATCH, num_swdge_queues=NQ
    )
    x_lo = nc.dram_tensor("x_lo", [SPLIT, PACK], mybir.dt.bfloat16, kind="ExternalInput")
    x_hi = nc.dram_tensor(
        "x_hi", [N_NODES - SPLIT, PACK], mybir.dt.bfloat16, kind="ExternalInput"
    )
    gg = nc.dram_tensor("gg", [128, 8 * C], mybir.dt.int16, kind="ExternalInput")
    dr = nc.dram_tensor("dr", [128, C], mybir.dt.bfloat16, kind="ExternalInput")
    iot = nc.dram_tensor("iot", [128, 128], mybir.dt.bfloat16, kind="ExternalInput")
    out = nc.dram_tensor("out", [OUT_ROWS, D], mybir.dt.float32, kind="ExternalOutput")

    with ExitStack() as stack:
        ec = stack.enter_context
        block = ec(nc.Block())
        msgs = [
            ec(nc.sbuf_tensor(f"msg{i}", [128, GCH, D], mybir.dt.bfloat16))
            for i in range(NBUF)
        ]
        inds = [
            ec(nc.sbuf_tensor(f"ind{i}", [128, GCH * 128], mybir.dt.bfloat16))
            for i in range(NBUF)
        ]
        t_gg = ec(nc.sbuf_tensor("t_gg", [128, 8 * C], mybir.dt.int16))
        t_dr = ec(nc.sbuf_tensor("t_dr", [128, C], mybir.dt.bfloat16))
        t_iot = ec(nc.sbuf_tensor("t_iot", [128, 128], mybir.dt.bfloat16))
        outbuf = ec(nc.sbuf_tensor("outbuf", [128, N_TILES * D], mybir.dt.float32))
        acc = ec(nc.psum_tensor("acc", [128, 4096], mybir.dt.float32))
        up = ec(nc.semaphore("up"))
        gsems = [ec(nc.semaphore(f"gs{i}")) for i in range(NBUF)]
        isem = ec(nc.semaphore("isem"))
        psem = ec(nc.semaphore("psem"))
        esem = ec(nc.semaphore("esem"))
        osem = ec(nc.semaphore("osem"))

        @block.sync
        def _(y):
            y.dma_start(t_gg[:, :], gg[:, :]).then_inc(up, 16)
            y.dma_start(t_dr[:, :], dr[:, :]).then_inc(up, 16)
            y.dma_start(t_iot[:, :], iot[:, :]).then_inc(up, 16)
            y.wait_ge(esem, N_BANKS)
            out_v = out[:, :].rearrange("(a p) d -> p a d", p=128)
            ob_v = outbuf[:, :].rearrange("p (a d) -> p a d", a=N_TILES)
            y.dma_start(out_v, ob_v).then_inc(osem, 16)
            y.wait_ge(osem, 16)

        @block.gpsimd
        def _(g):
            g.load_library(mlp)
            g.wait_ge(up, 16)  # t_gg uploaded
            for k, (tab, a, b) in enumerate(chunks):
                ncols = b - a
                n = 128 * ncols
                if k >= NBUF:
                    g.wait_ge(psem, k - NBUF + 1)
                table = x_lo if tab == 0 else x_hi
                _dma_gather_hi(
                    g,
                    nc,
                    msgs[k % NBUF][:, :ncols, :],
                    table[:, 0:D],
                    t_gg[:, 8 * a : 8 * b],
                    n,
                    k % NQ,
                ).then_inc(gsems[k % NBUF], 16)

        @block.vector
        def _(v):
            v.wait_ge(up, 48)
            for k, (tab, a, b) in enumerate(chunks):
                ncols = b - a
                if k >= NBUF:
                    v.wait_ge(psem, k - NBUF + 1)
                out_ap = inds[k % NBUF][:, : ncols * 128].rearrange(
                    "p (a b) -> p a b", a=ncols
                )
                dr_rep = (
                    t_dr[:, a:b].unsqueeze(2).broadcast_to([128, ncols, 128])
                )
                iot_rep = t_iot[:, :].unsqueeze(1).broadcast_to([128, ncols, 128])
                ins = v.tensor_tensor(
                    out_ap, dr_rep, iot_rep, mybir.AluOpType.is_equal
                )
                ins.then_inc(isem, 1)

        @block.tensor
        def _(t):
            nbuf_seen = [0] * NBUF
            for k, (tab, a, b) in enumerate(chunks):
                nbuf_seen[k % NBUF] += 1
                t.wait_ge(gsems[k % NBUF], 16 * nbuf_seen[k % NBUF])
                t.wait_ge(isem, k + 1)
                for j in range(b - a):
                    gidx = a + j
                    tl = int(group_tiles[gidx])
                    lhsT = inds[k % NBUF][:, 128 * j : 128 * (j + 1)]
                    mm = t.matmul(
                        acc[:, D * tl : D * (tl + 1)],
                        lhsT,
                        msgs[k % NBUF][:, j, :],
                        start=bool(starts[gidx]),
                        stop=bool(stops[gidx]),
                        skip_group_check=True,
                    )
                mm.then_inc(psem, 1)

        @block.scalar
        def _(s):
            for b in range(N_BANKS):
                s.wait_ge(psem, bank_done_chunk[b] + 1)
                t0, t1 = 8 * b, min(8 * b + 8, N_TILES)
                ins = s.copy(
                    outbuf[:, D * t0 : D * t1], acc[:, D * t0 : D * t1]
                )
                ins.then_inc(esem, 1)

    nc.compile()
    return nc


_NC_CACHE = {}


def _get_nc(meta):
    key = (
        meta["C"],
        meta["c_lo"],
        meta["group_tiles"].tobytes(),
        meta["starts"].tobytes(),
        meta["stops"].tobytes(),
    )
    if key not in _NC_CACHE:
        _NC_CACHE[key] = _build_nc(meta)
    return _NC_CACHE[key]


def _pack_table(x):
    hi = x.astype(BF16)
    lo = (x - hi.astype(np.float32)).astype(BF16)
    return np.ascontiguousarray(np.concatenate([hi, lo], axis=1))


def kernel_with_result(x, edge_index, trace=False):
    x = np.ascontiguousarray(np.asarray(x, dtype=np.float32))
    ei = np.asarray(edge_index)
    assert x.shape == (N_NODES, D), x.shape
    cores, meta = _build_layout(ei[0], ei[1])
    nc = _get_nc(meta)
    xp = _pack_table(x)
    x_lo = np.ascontiguousarray(xp[:SPLIT])
    x_hi = np.ascontiguousarray(xp[SPLIT:])
    iot = np.tile(np.arange(128, dtype=np.float32).astype(BF16), (128, 1))
    in_maps = [
        {
            "x_lo": x_lo,
            "x_hi": x_hi,
            "gg": _grid_to_wrapped(info["gg"]),
            "dr": np.ascontiguousarray(info["dr"].astype(BF16)),
            "iot": iot,
        }
        for info in cores
    ]
    res = run_bass_kernel_spmd(nc, in_maps, core_ids=list(range(N_CORES)), trace=trace)
    out = np.concatenate([r["out"][:NPC] for r in res.results], axis=0)
    return out, res


def kernel(x, edge_index):
    out, _ = kernel_with_result(x, edge_index)
    return out


# revision 9
# speedup vs baseline: 3.5016x; 1.1576x over previous
"""GNN message-passing (gather + segment_sum) Trainium2 kernel.

Reference semantics (full problem):
    out = segment_sum(x[src], dst, num_segments=50000)   x: [50000, 64] fp32
    edge_index: [2, 800000] (src; dst)

Sharding: destination nodes are range-partitioned over the 8 NeuronCores
(core c owns nodes [c*6250, (c+1)*6250)); each edge is routed to the core
owning its destination, so no cross-core reduction is needed. Each core
holds a full replica of the bf16 node-feature table in HBM (256 B rows;
only the first 128 B - the bf16 feature row - is fetched per edge).

Device algorithm per core:
  - gpsimd/SWDGE dma_gather: msg[i] = x_bf16[gather_idx[i]]  (HBM -> SBUF)
    emitted as a raw InstDMAGatherAnt with elem_size=64 bf16 (128 B) and
    elem_step=128 (256 B row stride): the non-transpose ucode path allows
    sub-256B elements as long as the row stride is a 256 B multiple.
    bf16 messages + fp32 PSUM accumulation give ~2e-3 relative error
    (graded gate is 2e-2). Gather chunks round-robin over all 4 SWDGE
    queues: each queue's descriptor generation runs on its own Q7 cpu
    pair, so desc-gen for different queues overlaps; a 6-deep buffer ring
    lets the next chunks' desc-gen overlap earlier chunks' transfers.
    Each buffer slot has its own DMA-completion semaphore: with two
    gathers of one queue in flight, a shared per-queue semaphore would
    alias their 16 per-engine completion increments.
  - vector/DVE: one batched is_equal per chunk builds the one-hot
    indicators for all its 128-edge groups at once:
        ind[p, j, n] = (dst_rel[p, a+j] == iota[n])   bf16 [128, ncols, 128]
    via stride-0 broadcast APs (dr replicated along n, iota along j).
    Dummy tokens carry dst_rel = -1 -> all-zero indicator row.
  - tensor/PE: per group, accumulate into the destination node-tile's PSUM
    accumulator (all 49 tiles of [128 nodes, 64] live in PSUM at once):
        psum[tile] += ind.T @ msg
  - scalar/ACT: evacuate each PSUM bank to SBUF as soon as its last group
    has accumulated; sync-engine HWDGE DMAs handle the input uploads and
    the final output store.

Host layout invariants (_build_layout):
  - edges sorted by (region = src<32768 ? lo : hi, dst tile, src); each
    (tile, region) block is padded to a multiple of 128 tokens so no
    128-token group spans two node tiles, and gather chunks within one
    region use a single table (int16 gather index limit).
  - per-(tile, region) column counts are maxed across all 8 cores so the
    SPMD instruction stream (PSUM offsets, start/stop flags) is identical.
  - token i of a dma_gather/SBUF grid lives at [i % 128, i // 128].
"""

from contextlib import ExitStack

import ml_dtypes
import numpy as np

import concourse.bacc as bacc
import concourse.mybir as mybir
from concourse.bass_utils import run_bass_kernel_spmd
from concourse.library_config import mlp

BF16 = ml_dtypes.bfloat16

N_NODES = 50000
N_EDGES = 800000
D = 64
PACK = 2 * D              # table row pitch: 128 bf16 = 256 B (only 128 B used)
N_CORES = 8
NPC = N_NODES // N_CORES  # 6250 destination nodes per core
N_TILES = (NPC + 127) // 128  # 49 node tiles per core
N_BANKS = (N_TILES + 7) // 8  # 7 PSUM banks hold the 49 tiles
OUT_ROWS = N_TILES * 128  # 6272
SPLIT = 32768             # int16 index limit for dma_gather
# SWDGE descriptor-ring capacity: ring_ndesc = scratch_bytes/16 per queue per
# side; a gather of n tokens needs n/16+1 descs per engine and must fit the
# ring with the in-flight chunks of that queue.
SCRATCH = 24576           # -> ring 1536 descs per queue
GCH = 56                  # max chunk size in grid columns
NQ = 4                    # SWDGE queues; chunk k runs on queue k % NQ
MBUF = 8                  # msg buffers (each with its own DMA-completion sem)
IBUF = 6                  # ind buffers


def _build_layout(src, dst):
    """Place edges on per-core token grids; uniform across cores.

    Returns (cores, meta) where cores[c] has:
      gg [128, C] int32 gather index grid (lo region: src; hi: src-SPLIT; pad 0)
      dr [128, C] float dst_rel grid (node index within tile; pad -1)
    and meta has:
      C, c_lo, group_tiles [C], starts [C], stops [C]
    """
    src = np.asarray(src, np.int64)
    dst = np.asarray(dst, np.int64)
    core_of = dst // NPC
    # per (core, region, tile) edge lists
    buckets = {}
    cols = np.zeros((2, N_TILES), np.int64)  # [region, tile] -> max cols
    for c in range(N_CORES):
        sel = core_of == c
        s = src[sel]
        d = dst[sel] - c * NPC
        tile = d // 128
        rel = d % 128
        hi = (s >= SPLIT).astype(np.int64)
        for r in (0, 1):
            rs = hi == r
            for t in range(N_TILES):
                m = rs & (tile == t)
                buckets[(c, r, t)] = (s[m], rel[m])
                cols[r, t] = max(cols[r, t], -(-int(m.sum()) // 128))
    c_lo = int(cols[0].sum())
    C = int(cols.sum())
    group_tiles = []
    for r in (0, 1):
        for t in range(N_TILES):
            group_tiles += [t] * int(cols[r, t])
    group_tiles = np.array(group_tiles, np.int64)
    # PSUM start=True clears the has_written bits of the WHOLE bank, so it may
    # only be issued once per bank (on the bank's first matmul). With the bit
    # clear, a start=False matmul overwrites-and-marks; with it set, it
    # accumulates -- exactly per-region init-then-accumulate semantics.
    starts = np.zeros(C, bool)
    stops = np.zeros(C, bool)
    group_banks = group_tiles // 8
    for b in range(int(group_banks.max()) + 1):
        w = np.nonzero(group_banks == b)[0]
        starts[w[0]] = True
        stops[w[-1]] = True
    cores = []
    for c in range(N_CORES):
        gg = np.zeros((128, C), np.int32)
        dr = np.full((128, C), -1.0, np.float32)
        col0 = 0
        for r in (0, 1):
            for t in range(N_TILES):
                s, rel = buckets[(c, r, t)]
                k = np.arange(len(s))
                p = k % 128
                col = col0 + k // 128
                gg[p, col] = s - (SPLIT if r else 0)
                dr[p, col] = rel
                col0 += int(cols[r, t])
        cores.append({"gg": gg, "dr": dr})
    meta = {
        "C": C,
        "c_lo": c_lo,
        "group_tiles": group_tiles,
        "starts": starts,
        "stops": stops,
    }
    return cores, meta


def _grid_to_wrapped(grid):
    """[128, C] token grid -> [128, C*8] int16 wrapped index array.

    Token i lives at grid[i % 128, i // 128]; the SWDGE ucode reads token i
    from wrapped[i % 16, i // 16], replicated to all 8 Q7 cpu partition
    groups (rows 16k..16k+15) so any queue's cpu pair finds them.
    """
    P, C = grid.shape
    assert P == 128
    tok = grid.T.reshape(-1)
    return np.tile(tok.reshape(-1, 16).T.astype(np.int16), (8, 1))


def _chunks(meta):
    """Balanced chunks of <= GCH columns, not crossing the lo/hi boundary.

    Returns list of (table_idx, col_a, col_b)."""
    out = []
    for lo, hi_, tab in ((0, meta["c_lo"], 0), (meta["c_lo"], meta["C"], 1)):
        n = hi_ - lo
        k = -(-n // GCH)
        edges = [lo + (n * i) // k for i in range(k)] + [hi_]
        for i in range(k):
            out.append((tab, edges[i], edges[i + 1]))
    return out


def _dma_gather_hi(g, nc, out_ap, in_ap, idxs_ap, num_idxs, queue_num):
    """Raw InstDMAGatherAnt: gather elem_size=64 bf16 (128 B) rows from a
    table with 256 B row pitch (elem_step=128). Mirrors bass.dma_gather's
    non-transpose DRAM lowering, which over-conservatively asserts
    elem%256B: the ucode only requires the row *stride* be a 256 B
    multiple (the %256 assert guards the transpose path)."""
    elem_size = D          # 64 bf16 = 128 B fetched per token
    elem_step = PACK       # 256 B row pitch
    stride_bytes = elem_step * 2
    assert stride_bytes % 256 == 0
    assert in_ap.ap[0][0] == elem_step, in_ap.ap
    assert in_ap.ap[-1][1] == elem_size, in_ap.ap
    assert out_ap.ap[-1][1] == elem_size, out_ap.ap
    assert out_ap.ap[0][1] * out_ap.ap[1][1] == num_idxs
    assert num_idxs % 128 == 0
    _in_ap = g.lower_ap_dma(in_ap, for_custom_bir_dma=True)
    _idxs_ap = g.lower_ap(idxs_ap)
    _out_ap = g.lower_ap(out_ap)
    return g.add_instruction(
        mybir.InstDMAGatherAnt(
            name=nc.get_next_instruction_name(),
            ins=[*_in_ap, _idxs_ap, g.lower_val_access(g.to_reg(num_idxs))],
            outs=[_out_ap],
            transpose=False,
            num_idxs=num_idxs,
            elem_size=elem_size,
            stride_bytes_256=stride_bytes // 256,
            gen_mode=0,
            single_packet=False,
            queue_num=queue_num,
            sbuf_tokens_per_rank=0,
            sbuf_free_dim_per_rank=0,
            sbuf_free_dim_pad_per_rank=0,
            sbuf_byte_offset=0,
        )
    )


def _build_nc(meta):
    C = meta["C"]
    chunks = _chunks(meta)
    n_ch = len(chunks)
    group_tiles = meta["group_tiles"]
    starts = meta["starts"]
    stops = meta["stops"]
    # chunk index after which PSUM bank b is complete (its stop group done)
    bank_done_chunk = [0] * N_BANKS
    for b in range(N_BANKS):
        w = np.nonzero(group_tiles // 8 == b)[0]
        last = int(w[-1])
        for k, (_, a, bb) in enumerate(chunks):
            if a <= last < bb:
                bank_done_chunk[b] = k
                break

    nc = bacc.Bacc(
        "TRN2", dynamic_dma_scratch_size=SCRATCH, num_swdge_queues=NQ
    )
    x_lo = nc.dram_tensor("x_lo", [SPLIT, PACK], mybir.dt.bfloat16, kind="ExternalInput")
    x_hi = nc.dram_tensor(
        "x_hi", [N_NODES - SPLIT, PACK], mybir.dt.bfloat16, kind="ExternalInput"
    )
    gg = nc.dram_tensor("gg", [128, 8 * C], mybir.dt.int16, kind="ExternalInput")
    dr = nc.dram_tensor("dr", [128, C], mybir.dt.bfloat16, kind="ExternalInput")
    iot = nc.dram_tensor("iot", [128, 128], mybir.dt.bfloat16, kind="ExternalInput")
    out = nc.dram_tensor("out", [OUT_ROWS, D], mybir.dt.float32, kind="ExternalOutput")

    with ExitStack() as stack:
        ec = stack.enter_context
        block = ec(nc.Block())
        msgs = [
            ec(nc.sbuf_tensor(f"msg{i}", [128, GCH, D], mybir.dt.bfloat16))
            for i in range(MBUF)
        ]
        inds = [
            ec(nc.sbuf_tensor(f"ind{i}", [128, GCH * 128], mybir.dt.bfloat16))
            for i in range(IBUF)
        ]
        t_gg = ec(nc.sbuf_tensor("t_gg", [128, 8 * C], mybir.dt.int16))
        t_dr = ec(nc.sbuf_tensor("t_dr", [128, C], mybir.dt.bfloat16))
        t_iot = ec(nc.sbuf_tensor("t_iot", [128, 128], mybir.dt.bfloat16))
        outbuf = ec(nc.sbuf_tensor("outbuf", [128, N_TILES * D], mybir.dt.float32))
        acc = ec(nc.psum_tensor("acc", [128, 4096], mybir.dt.float32))
        up = ec(nc.semaphore("up"))
        gsems = [ec(nc.semaphore(f"gs{i}")) for i in range(MBUF)]
        isem = ec(nc.semaphore("isem"))
        psem = ec(nc.semaphore("psem"))
        esem = ec(nc.semaphore("esem"))
        osem = ec(nc.semaphore("osem"))

        @block.sync
        def _(y):
            y.dma_start(t_gg[:, :], gg[:, :]).then_inc(up, 16)
            y.dma_start(t_dr[:, :], dr[:, :]).then_inc(up, 16)
            y.dma_start(t_iot[:, :], iot[:, :]).then_inc(up, 16)
            out_v = out[:, :].rearrange("(a p) d -> p a d", p=128)
            ob_v = outbuf[:, :].rearrange("p (a d) -> p a d", a=N_TILES)
            for b in range(N_BANKS):
                y.wait_ge(esem, b + 1)
                t0, t1 = 8 * b, min(8 * b + 8, N_TILES)
                y.dma_start(out_v[:, t0:t1, :], ob_v[:, t0:t1, :]).then_inc(
                    osem, 16
                )
            y.wait_ge(osem, 16 * N_BANKS)

        @block.gpsimd
        def _(g):
            g.load_library(mlp)
            g.wait_ge(up, 16)  # t_gg uploaded
            for k, (tab, a, b) in enumerate(chunks):
                ncols = b - a
                n = 128 * ncols
                if k >= MBUF:
                    g.wait_ge(psem, k - MBUF + 1)
                table = x_lo if tab == 0 else x_hi
                _dma_gather_hi(
                    g,
                    nc,
                    msgs[k % MBUF][:, :ncols, :],
                    table[:, 0:D],
                    t_gg[:, 8 * a : 8 * b],
                    n,
                    k % NQ,
                ).then_inc(gsems[k % MBUF], 16)

        @block.vector
        def _(v):
            v.wait_ge(up, 48)
            for k, (tab, a, b) in enumerate(chunks):
                ncols = b - a
                if k >= IBUF:
                    v.wait_ge(psem, k - IBUF + 1)
                out_ap = inds[k % IBUF][:, : ncols * 128].rearrange(
                    "p (a b) -> p a b", a=ncols
                )
                dr_rep = (
                    t_dr[:, a:b].unsqueeze(2).broadcast_to([128, ncols, 128])
                )
                iot_rep = t_iot[:, :].unsqueeze(1).broadcast_to([128, ncols, 128])
                ins = v.tensor_tensor(
                    out_ap, dr_rep, iot_rep, mybir.AluOpType.is_equal
                )
                ins.then_inc(isem, 1)

        @block.tensor
        def _(t):
            nbuf_seen = [0] * MBUF
            for k, (tab, a, b) in enumerate(chunks):
                nbuf_seen[k % MBUF] += 1
                t.wait_ge(gsems[k % MBUF], 16 * nbuf_seen[k % MBUF])
                t.wait_ge(isem, k + 1)
                for j in range(b - a):
                    gidx = a + j
                    tl = int(group_tiles[gidx])
                    lhsT = inds[k % IBUF][:, 128 * j : 128 * (j + 1)]
                    mm = t.matmul(
                        acc[:, D * tl : D * (tl + 1)],
                        lhsT,
                        msgs[k % MBUF][:, j, :],
                        start=bool(starts[gidx]),
                        stop=bool(stops[gidx]),
                        skip_group_check=True,
                    )
                mm.then_inc(psem, 1)

        @block.scalar
        def _(s):
            for b in range(N_BANKS):
                s.wait_ge(psem, bank_done_chunk[b] + 1)
                t0, t1 = 8 * b, min(8 * b + 8, N_TILES)
                ins = s.copy(
                    outbuf[:, D * t0 : D * t1], acc[:, D * t0 : D * t1]
                )
                ins.then_inc(esem, 1)

    nc.compile()
    return nc


_NC_CACHE = {}


def _get_nc(meta):
    key = (
        meta["C"],
        meta["c_lo"],
        meta["group_tiles"].tobytes(),
        meta["starts"].tobytes(),
        meta["stops"].tobytes(),
    )
    if key not in _NC_CACHE:
        _NC_CACHE[key] = _build_nc(meta)
    return _NC_CACHE[key]


def _pack_table(x):
    hi = x.astype(BF16)
    lo = (x - hi.astype(np.float32)).astype(BF16)
    return np.ascontiguousarray(np.concatenate([hi, lo], axis=1))


def kernel_with_result(x, edge_index, trace=False):
    x = np.ascontiguousarray(np.asarray(x, dtype=np.float32))
    ei = np.asarray(edge_index)
    assert x.shape == (N_NODES, D), x.shape
    cores, meta = _build_layout(ei[0], ei[1])
    nc = _get_nc(meta)
    xp = _pack_table(x)
    x_lo = np.ascontiguousarray(xp[:SPLIT])
    x_hi = np.ascontiguousarray(xp[SPLIT:])
    iot = np.tile(np.arange(128, dtype=np.float32).astype(BF16), (128, 1))
    in_maps = [
        {
            "x_lo": x_lo,
            "x_hi": x_hi,
            "gg": _grid_to_wrapped(info["gg"]),
            "dr": np.ascontiguousarray(info["dr"].astype(BF16)),
            "iot": iot,
        }
        for info in cores
    ]
    res = run_bass_kernel_spmd(nc, in_maps, core_ids=list(range(N_CORES)), trace=trace)
    out = np.concatenate([r["out"][:NPC] for r in res.results], axis=0)
    return out, res


def kernel(x, edge_index):
    out, _ = kernel_with_result(x, edge_index)
    return out
